# revision 25
# baseline (speedup 1.0000x reference)
"""Trainium2 Bass kernel for the digit-conv model.

Math: y = relu(relu(conv3x3(x) @ W1 + b1) @ W2 + b2) @ W3 + b3.
The valid 3x3 conv is linear, so it folds into W1 on device:
feat = x @ A with A[u, q] sparse from conv_w, hence
W1eff = A @ W1 and y = mlp(x @ W1eff ...). The kernel computes
W1eff = A^T.T @ W1 on the tensor engine once (A^T is banded, so
all-zero blocks are statically skipped), then streams the batch
through the 3-layer MLP entirely as lhsT.T @ rhs matmuls with channels
on partitions and batch on the free dimension.

Sharding: pure data parallel - batch split across 8 cores, weights
replicated. Host-side work is limited to layout (x transpose + shard +
bf16 cast, zero-padding, band extraction, pre-tiling) and scattering
the 9 conv weights into the A^T matrix (no arithmetic).

PE array packing (measured on HW via tile_position microbenchmarks:
tiled matmuls in disjoint 32-strips overlap with delta ~3ns, mode
switch ~105ns):
  - The 784-long L1 contraction is tiled as 6x128 + a 16-row edge.
    x ships with a 128-row "slab" chunk holding the edge rows
    replicated at partition strips 0/32/64/96.
  - L1 output chunks are [128, 128, 44]. Per block pair, the 24 full
    (128,128) passes are followed by ONE (32,128) row-tiled slot with
    the 4 edge passes (blocks a/b x chunks 0/1) running concurrently
    at strips 0/32/64/96.
  - The 44-chunk of both blocks runs as a column-tiled (128,64) pair
    (block a at PSUM 0:44 / array cols 0:63, block b at 64:108 /
    64:127): 6 full-k pairs + 1 edge pair (edge weights zero-padded to
    128 contraction rows so the mode stays (128,64)). h1's third chunk
    for "b" blocks stays at partitions 64:108; bias/W2 rows are
    replicated there host-side so downstream ops stay lane-locked.
  - L2's third k-pass (contraction 44) for blocks a/b runs as a
    row-tiled (64,128) pair at row strips 0/64 - one slot.
  - W3 is zero-padded [100, 112] so L3 runs in full (128,128) mode.
Per 1024-column pair this is 39 PSUM-bank passes vs 50 for the naive
3x100-chunk schedule; L1 array utilization ~90%.

All matmul operands are bf16; PSUM accumulation is fp32; biases are
applied in fp32 from PSUM.

DMA: cost is dominated by element (per-partition-line) count, so all
large tensors are pre-tiled on host into [partitions, contiguous-line]
layouts (x: one contiguous [128, 7*sw] block per super). x and the
fold band ride the Sync HWDGE queue; the fold w1 goes on the Scalar
HWDGE queue (issued before any ACT work so it isn't trapped behind
compute); w2/w3/bias on the GpSimd SWDGE path. Super widths ramp
256,256,512,512 -> 1024 steady so the inflow stays ahead of the PE and
the HAM clock-gate never re-throttles mid-stream; the final 384+128
supers shorten the end-of-stream dependency chain (last block solo,
128 wide). A dummy activation right after the first warmup matmul
forces the lazy ACT_TABLE_LOAD (~1.3us) into the fold-DMA shadow.
"""

import ml_dtypes
import numpy as np

import concourse.tile as tile
from concourse import bacc, mybir
from concourse import bass_utils

N_CORES = 8
B = 65536
BC = B // N_CORES  # 8192 rows per core
U = 784            # input features (28*28)
Q = 676            # conv outputs (26*26)
QP = 768           # q padded to 6 full tiles of 128
H1, H2, H3 = 300, 100, 10
W3P = 112          # w3 padded output width (keeps L3 in full PE mode)
NB = 512           # batch columns per PSUM block (one bank of fp32)
SUP = 1024         # max batch columns per DMA super-block
SUP_WIDTHS = [256, 256, 512, 512, 1024, 1024, 1024, 1024, 1024, 1024,
              384, 128]
assert sum(SUP_WIDTHS) == BC
KT = 128           # u-dim k-tile
NKF = 6            # full k-tiles (6*128 = 768)
NKT = 7            # + the edge slab chunk
UE = 768           # edge rows start (u 768:784, 16 rows)
# L1 output chunks: (start, size); the 44-chunk is the col-tiled one
MCS = [(0, 128), (128, 128), (256, 44)]
ABW = 384          # amat band width (3 u-chunks of 128)

_prog_cache = {}


def _fold_bands():
    """Static block-sparsity of A^T [Q, U]: per 128-row q-tile, the
    nonzero columns lie in a band; returns per-tile (q0, p_real, c_lo,
    c_hi) with the band given in whole 128-wide u-chunks."""
    bands = []
    for qt in range(QP // 128):
        q0 = qt * 128
        p_real = min(128, Q - q0)
        i_lo = q0 // 26
        i_hi = (q0 + p_real - 1) // 26
        u_lo = 28 * i_lo
        u_hi = min(U, 28 * (i_hi + 2) + 28)   # exclusive upper bound
        c_lo = u_lo // KT
        c_hi = (u_hi + KT - 1) // KT          # exclusive chunk bound
        assert c_hi - c_lo <= ABW // KT
        bands.append((q0, p_real, c_lo, c_hi))
    return bands


def _build_program():
    f32 = mybir.dt.float32
    bf16 = mybir.dt.bfloat16
    relu = mybir.ActivationFunctionType.Relu
    alu_add = mybir.AluOpType.add
    alu_max = mybir.AluOpType.max

    nc = bacc.Bacc(
        "TRN2", target_bir_lowering=False, debug=False, num_devices=N_CORES
    )

    nqt = QP // 128
    xT_d = nc.dram_tensor("xT", [128, NKT * BC], bf16,
                          kind="ExternalInput").ap()
    fband_d = nc.dram_tensor("fband", [128, nqt * ABW], bf16,
                             kind="ExternalInput").ap()
    fw1_d = nc.dram_tensor("fw1", [128, nqt * H1], bf16,
                           kind="ExternalInput").ap()
    w2_d = nc.dram_tensor("w2", [128, 3 * H2], bf16, kind="ExternalInput").ap()
    w3_d = nc.dram_tensor("w3", [H2, W3P], bf16, kind="ExternalInput").ap()
    bias_d = nc.dram_tensor("bias", [128, 5], f32, kind="ExternalInput").ap()
    yT_d = nc.dram_tensor("yT", [H3, BC], f32, kind="ExternalOutput").ap()

    bands = _fold_bands()

    with tile.TileContext(nc) as tc:
        with tc.tile_pool(name="const", bufs=1) as cpool, \
             tc.tile_pool(name="xp", bufs=5) as xpool, \
             tc.tile_pool(name="h1p", bufs=4) as h1pool, \
             tc.tile_pool(name="h2p", bufs=2) as h2pool, \
             tc.tile_pool(name="yp", bufs=2) as ypool, \
             tc.tile_pool(name="ps1", bufs=6, space="PSUM") as ps1p, \
             tc.tile_pool(name="ps2", bufs=2, space="PSUM") as ps2p:

            # ---- constants into SBUF. The fold is split in two DMAs on
            # the two HWDGE queues (band on Sync, w1 on Scalar) so the
            # transfers overlap; w2/w3/bias ride the GpSimd SWDGE path ----
            warm_sb = cpool.tile([128, 512], bf16)
            nc.gpsimd.memset(warm_sb[:], 0.0)
            zb_sb = cpool.tile([128, 1], f32)
            nc.gpsimd.memset(zb_sb[:], 0.0)
            scr_sb = cpool.tile([128, 16], bf16)

            fold_sb = cpool.tile([128, nqt * (ABW + H1)], bf16)
            W1OFF = nqt * ABW  # w1 bands start here
            # halves: the fold matmuls for low u-chunks only need q-tiles
            # 0..2, so they start as soon as the first halves land
            nc.sync.dma_start(fold_sb[:, :3 * ABW], fband_d[:, :3 * ABW])
            nc.scalar.dma_start(fold_sb[:, W1OFF:W1OFF + 3 * H1],
                                fw1_d[:, :3 * H1])
            nc.sync.dma_start(fold_sb[:, 3 * ABW:W1OFF],
                              fband_d[:, 3 * ABW:])
            nc.scalar.dma_start(fold_sb[:, W1OFF + 3 * H1:],
                                fw1_d[:, 3 * H1:])

            # ---- HAM warmup: dummy matmuls so the PE clock-gate starts
            # releasing before the fold lands (the fold matmuls finish
            # the warmup as real work). All dummies share one PSUM tile
            # (WAW on the in-order PE queue needs no semaphores). The
            # dummy activation forces the lazy ACT_TABLE_LOAD here ----
            pw = ps2p.tile([128, NB], f32, tag="l2", name="pwarm")
            for wi in range(5):
                nc.tensor.matmul(pw[:], warm_sb[:, :128], warm_sb[:],
                                 start=True, stop=True)

            w2_sb = cpool.tile([128, 3 * H2], bf16)
            nc.gpsimd.dma_start(w2_sb[:], w2_d)
            w3_sb = cpool.tile([H2, W3P], bf16)
            nc.gpsimd.dma_start(w3_sb[:], w3_d)
            bias_sb = cpool.tile([128, 5], f32)
            nc.gpsimd.dma_start(bias_sb[:], bias_d)

            # ---- fold the conv into W1: W1eff[u, c] = (A^T).T @ W1 ----
            # ut 0..5 produce the full 128-row chunks; ut 6 produces the
            # 16-row edge (only qt5's band reaches u>=768; the band
            # columns past u=784 are zero so rows 16:128 come out zero).
            w1eff_sb = cpool.tile([128, NKF * H1], bf16)
            w1edge_sb = cpool.tile([112, 256], bf16)
            w1e44_sb = cpool.tile([128, 44], bf16)
            for ut in range(NKF + 1):
                parts = [qt for qt, (_, _, c_lo, c_hi) in enumerate(bands)
                         if c_lo <= ut < c_hi]
                assert parts
                pf = ps1p.tile([128, NB], f32, tag="l1", name=f"pfold_{ut}")
                for idx, qt in enumerate(parts):
                    _, _, c_lo, _ = bands[qt]
                    off = qt * ABW + (ut - c_lo) * KT
                    nc.tensor.matmul(
                        pf[:, :H1],
                        fold_sb[:, off:off + KT],
                        fold_sb[:, W1OFF + qt * H1:W1OFF + (qt + 1) * H1],
                        start=(idx == 0),
                        stop=(idx == len(parts) - 1),
                    )
                if ut < NKF:
                    nc.vector.tensor_copy(
                        w1eff_sb[:, ut * H1:(ut + 1) * H1], pf[:, :H1])
                else:
                    # edge weights, replicated for the row-tiled strips:
                    # chunk0 at strips 0/64, chunk1 at strips 32/96, and
                    # the 44-chunk zero-padded to 128 contraction rows
                    nc.vector.tensor_copy(w1edge_sb[0:16, 0:128],
                                          pf[0:16, 0:128])
                    nc.vector.tensor_copy(w1edge_sb[64:80, 0:128],
                                          pf[0:16, 0:128])
                    nc.vector.tensor_copy(w1edge_sb[32:48, 128:256],
                                          pf[0:16, 128:256])
                    nc.vector.tensor_copy(w1edge_sb[96:112, 128:256],
                                          pf[0:16, 128:256])
                    nc.vector.tensor_copy(w1e44_sb[:], pf[:, 256:300])

            # ---- post-fold filler: bridge any fold->x_0 DMA wait so an
            # unlucky HAM MID-window phase can't re-throttle the PE. The
            # dummy activation lives here (not at the warmup) so the lazy
            # ACT_TABLE_LOAD the compiler puts before the first ACTIVATE
            # cannot delay the fw1 dma issue at the head of the Scalar
            # queue ----
            for wi in range(2):
                nc.tensor.matmul(pw[:], warm_sb[:, :128], warm_sb[:],
                                 start=True, stop=True)
                if wi == 0:
                    nc.scalar.activation(scr_sb[:], pw[:, :16], relu,
                                         bias=zb_sb[:, 0:1], scale=1.0)

            # ---- main pipeline over batch super-blocks ----
            # Blocks are processed in PAIRS (a, b); L2/L3 of each group
            # are emitted AFTER the next group's L1, so the PE reaches
            # them with their ACT dependencies long satisfied.
            def w1slice(kt, mc):
                c0, csz = MCS[mc]
                return w1eff_sb[:, kt * H1 + c0:kt * H1 + c0 + csz]

            def xsl(blk, kt):
                xtile, sw, pb, nb = (blk["xt"], blk["sw"], blk["pb"],
                                     blk["nb"])
                return xtile[:, kt * sw + pb * NB:kt * sw + pb * NB + nb]

            def emit_chunk01(blk, mc):
                """Six full-mode L1 passes for chunk 0/1 of one block;
                the 16-row edge pass and the relu come later (grouped)."""
                p1 = ps1p.tile([128, blk["nb"]], f32, tag="l1",
                               name=f"p1_{blk['id']}_{mc}",
                               padded_shape=[128, NB])
                blk[f"p1_{mc}"] = p1
                for kt in range(NKF):
                    nc.tensor.matmul(
                        p1[:], w1slice(kt, mc), xsl(blk, kt),
                        start=(kt == 0), stop=False,
                    )

            def emit_edge_acts(group):
                """One (32,128) row-tiled slot: the 4 edge passes of the
                pair (blocks a/b x chunks 0/1) run concurrently at strips
                0/32/64/96, each closing its bank's accumulation; then
                the four relu ACTs."""
                strips = [(0, 32), (64, 96)]
                for blk, (r0, r1) in zip(group, strips):
                    xtile, sw, pb, nb = (blk["xt"], blk["sw"], blk["pb"],
                                         blk["nb"])
                    for mc, r in ((0, r0), (1, r1)):
                        nc.tensor.matmul(
                            blk[f"p1_{mc}"][:],
                            w1edge_sb[r:r + 16, mc * 128:(mc + 1) * 128],
                            xtile[r:r + 16,
                                  NKF * sw + pb * NB:NKF * sw + pb * NB + nb],
                            start=False, stop=True,
                            tile_position=(r, 0),
                        )
                for blk in group:
                    nb = blk["nb"]
                    for mc in (0, 1):
                        nc.scalar.activation(
                            blk["h1"][:, mc * nb:(mc + 1) * nb],
                            blk[f"p1_{mc}"][:], relu,
                            bias=bias_sb[:, mc:mc + 1], scale=1.0,
                        )

            def emit_m2(group):
                """Col-tiled (128,64) passes for the 44-chunk: block a at
                PSUM 0:44 / array cols 0:63, block b at 64:108 / 64:127.
                6 full-k pairs plus one edge pair (edge weights padded to
                128 contraction rows). A solo block only uses side a."""
                p1c = ps1p.tile([128, NB], f32, tag="l1",
                                name=f"p1c_{group[0]['id']}")
                sides = [(0, 44), (64, 108)]
                for kt in range(NKF):
                    for blk, (s0, s1) in zip(group, sides):
                        nc.tensor.matmul(
                            p1c[s0:s1, :blk["nb"]], w1slice(kt, 2),
                            xsl(blk, kt), start=(kt == 0), stop=False,
                        )
                for blk, (s0, s1) in zip(group, sides):
                    xtile, sw, pb, nb = (blk["xt"], blk["sw"], blk["pb"],
                                         blk["nb"])
                    nc.tensor.matmul(
                        p1c[s0:s1, :nb],
                        w1e44_sb[:, :],
                        xtile[:, NKF * sw + pb * NB:NKF * sw + pb * NB + nb],
                        start=False, stop=True,
                    )
                for blk, (s0, s1) in zip(group, sides):
                    nb = blk["nb"]
                    blk["h1s"] = s0
                    nc.scalar.activation(
                        blk["h1"][s0:s1, 2 * nb:3 * nb], p1c[s0:s1, :nb],
                        relu, bias=bias_sb[s0:s1, 2:3], scale=1.0,
                    )

            def emit_l2l3(group):
                """L2 + L3 + output for a group of 1-2 blocks. The two
                full L2 k-passes per block are full-mode; the
                44-contraction pass runs as a row-tiled (64,128) pair at
                strips 0/64."""
                p2 = []
                for blk in group:
                    p2.append(ps2p.tile([H2, blk["nb"]], f32, tag="l2",
                                        name=f"p2_{blk['id']}",
                                        padded_shape=[H2, NB]))
                # the row-tiled 44-contraction pair runs FIRST (start) so
                # each bank's accumulation closes on the k1 pass and the
                # h2 post-processing starts two slots earlier
                for blk, p in zip(group, p2):
                    nb = blk["nb"]
                    s0 = blk["h1s"]
                    nc.tensor.matmul(
                        p[:], w2_sb[s0:s0 + 44, 2 * H2:3 * H2],
                        blk["h1"][s0:s0 + 44, 2 * nb:3 * nb],
                        start=True, stop=False,
                    )
                for k2 in range(2):
                    for blk, p in zip(group, p2):
                        nb = blk["nb"]
                        nc.tensor.matmul(
                            p[:], w2_sb[:, k2 * H2:(k2 + 1) * H2],
                            blk["h1"][:, k2 * nb:(k2 + 1) * nb],
                            start=False, stop=(k2 == 1),
                        )
                # h2 relu+bias: block a on DVE, block b on ACT, so the two
                # run concurrently and neither L3 stalls the PE queue long
                h2s = []
                for gi, (blk, p) in enumerate(zip(group, p2)):
                    h2 = h2pool.tile([H2, blk["nb"]], bf16, tag="h2",
                                     name=f"h2_{blk['id']}",
                                     padded_shape=[H2, NB])
                    if gi == 0:
                        nc.vector.tensor_scalar(
                            h2[:], p[:], bias_sb[:H2, 3:4], 0.0,
                            alu_add, alu_max)
                    else:
                        nc.scalar.activation(
                            h2[:], p[:], relu,
                            bias=bias_sb[:H2, 3:4], scale=1.0)
                    h2s.append(h2)
                for blk, h2 in zip(group, h2s):
                    nb = blk["nb"]
                    p3 = ps2p.tile([W3P, nb], f32, tag="l2",
                                   name=f"p3_{blk['id']}",
                                   padded_shape=[W3P, NB])
                    nc.tensor.matmul(p3[:], w3_sb[:], h2[:],
                                     start=True, stop=True)
                    nc.vector.tensor_scalar_add(
                        blk["y"][:, blk["pb"] * NB:blk["pb"] * NB + nb],
                        p3[:H3, :], bias_sb[:H3, 4:5])
                    if blk["last"]:
                        nc.sync.dma_start(
                            yT_d[:, blk["y0"]:blk["y0"] + blk["sw"]],
                            blk["y"][:])

            pending = None
            open_blk = None
            sup_start = 0
            last_sup = len(SUP_WIDTHS) - 1
            for sup, sw in enumerate(SUP_WIDTHS):
                xtile = xpool.tile([128, NKT * sw], bf16, tag="x",
                                   name=f"xt_{sup}",
                                   padded_shape=[128, NKT * SUP])
                nc.sync.dma_start(
                    xtile[:],
                    xT_d[:, NKT * sup_start:NKT * (sup_start + sw)],
                )
                y_sb = ypool.tile([H3, sw], f32, tag="y", name=f"y_{sup}",
                                  padded_shape=[H3, SUP])
                nblocks = (sw + NB - 1) // NB
                for pb in range(nblocks):
                    nb = min(NB, sw - pb * NB)
                    h1 = h1pool.tile([128, 3 * nb], bf16, tag="h1",
                                     name=f"h1_{sup}_{pb}",
                                     padded_shape=[128, 3 * NB])
                    blk = {"xt": xtile, "sw": sw, "pb": pb, "nb": nb,
                           "h1": h1, "y": y_sb, "y0": sup_start,
                           "last": pb == nblocks - 1, "id": f"{sup}_{pb}"}
                    emit_chunk01(blk, 0)
                    emit_chunk01(blk, 1)
                    if open_blk is None and sup == last_sup:
                        emit_edge_acts([blk])
                        emit_m2([blk])
                        if pending is not None:
                            emit_l2l3(pending)
                        pending = [blk]
                    elif open_blk is None:
                        open_blk = blk
                    else:
                        grp = [open_blk, blk]
                        emit_edge_acts(grp)
                        emit_m2(grp)
                        if pending is not None:
                            emit_l2l3(pending)
                        pending = grp
                        open_blk = None
                sup_start += sw
            assert open_blk is None
            emit_l2l3(pending)

    nc.compile()
    return nc


def _build_amat_banded(conv_w: np.ndarray) -> np.ndarray:
    """Scatter the 9 conv weights into the banded A^T [QP, ABW]:
    A^T[q, u] = conv_w[ki, kj] for q = 26*i + j, u = 28*(i+ki) + (j+kj),
    stored per 128-row q-tile with columns [c_lo*KT, c_hi*KT) of the
    band (clipped at u=784; the rest stays zero)."""
    amat = np.zeros((Q, U), np.float32)
    i = np.arange(26)
    j = np.arange(26)
    q = (26 * i[:, None] + j[None, :]).ravel()
    for ki in range(3):
        for kj in range(3):
            u = (28 * (i[:, None] + ki) + j[None, :] + kj).ravel()
            amat[q, u] = conv_w[ki, kj]
    banded = np.zeros((QP, ABW), np.float32)
    for (q0, p_real, c_lo, c_hi) in _fold_bands():
        w = min(c_hi * KT, U) - c_lo * KT
        banded[q0:q0 + p_real, :w] = amat[q0:q0 + p_real,
                                          c_lo * KT:c_lo * KT + w]
    return banded


def _make_in_maps(x, conv_w, W1, b1, W2, b2, W3, b3):
    bf = ml_dtypes.bfloat16
    xT = np.ascontiguousarray(x.T.astype(bf))  # [U, B] bf16
    fband = _build_amat_banded(conv_w).astype(bf)
    fband = np.ascontiguousarray(
        fband.reshape(QP // 128, 128, ABW).transpose(1, 0, 2)
        .reshape(128, -1))
    fw1 = np.zeros((QP, H1), np.float32)
    fw1[:Q] = np.asarray(W1, np.float32)
    fw1 = np.ascontiguousarray(
        fw1.astype(bf).reshape(QP // 128, 128, H1).transpose(1, 0, 2)
        .reshape(128, -1))
    # w2 packed as 3 k-chunk bands [128, 100] each; the 44-row third chunk
    # is replicated at partitions 64:108 for the "b" blocks of each pair
    W2f = np.asarray(W2, np.float32)
    w2pk = np.zeros((128, 3 * H2), np.float32)
    w2pk[:, 0:H2] = W2f[0:128]
    w2pk[:, H2:2 * H2] = W2f[128:256]
    w2pk[0:44, 2 * H2:3 * H2] = W2f[256:300]
    w2pk[64:108, 2 * H2:3 * H2] = W2f[256:300]
    w2pk = np.ascontiguousarray(w2pk.astype(bf))
    # w3 zero-padded [100, 112] so L3 stays in full (128,128) PE mode
    w3pk = np.zeros((H2, W3P), np.float32)
    w3pk[:, :H3] = np.asarray(W3, np.float32)
    w3pk = np.ascontiguousarray(w3pk.astype(bf))
    b1f = np.asarray(b1, np.float32)
    bias = np.zeros((128, 5), np.float32)
    bias[0:128, 0] = b1f[0:128]
    bias[0:128, 1] = b1f[128:256]
    bias[0:44, 2] = b1f[256:300]
    bias[64:108, 2] = b1f[256:300]
    bias[:H2, 3] = np.asarray(b2, np.float32)
    bias[:H3, 4] = np.asarray(b3, np.float32)
    in_maps = []
    for c in range(N_CORES):
        # x per core: 6 full 128-row u-chunks + a 128-row slab with the
        # 16 edge rows (u 768:784) replicated at strips 0/32/64/96, then
        # pre-tiled to one contiguous [128, 7*sw] block per super
        xc = xT[:, c * BC:(c + 1) * BC]
        xp = np.zeros((NKT * 128, BC), ml_dtypes.bfloat16)
        xp[:UE] = xc[:UE]
        for r in range(4):
            xp[UE + 32 * r:UE + 32 * r + 16] = xc[UE:U]
        xr = xp.reshape(NKT, 128, BC)
        xtt = np.empty((128, NKT * BC), ml_dtypes.bfloat16)
        s0 = 0
        for sw in SUP_WIDTHS:
            xtt[:, NKT * s0:NKT * (s0 + sw)] = (
                xr[:, :, s0:s0 + sw].transpose(1, 0, 2).reshape(128, -1))
            s0 += sw
        in_maps.append({
            "xT": np.ascontiguousarray(xtt),
            "fband": fband, "fw1": fw1,
            "w2": w2pk, "w3": w3pk,
            "bias": bias,
        })
    return in_maps


def kernel(x, conv_w, W1, b1, W2, b2, W3, b3):
    x = np.asarray(x, dtype=np.float32)
    conv_w = np.asarray(conv_w, dtype=np.float32)

    if "nc" not in _prog_cache:
        _prog_cache["nc"] = _build_program()
    nc = _prog_cache["nc"]

    in_maps = _make_in_maps(x, conv_w, W1, b1, W2, b2, W3, b3)
    res = bass_utils.run_bass_kernel_spmd(
        nc, in_maps, core_ids=list(range(N_CORES))
    )

    out = np.empty((B, H3), np.float32)
    for c in range(N_CORES):
        out[c * BC:(c + 1) * BC, :] = res.results[c]["yT"].T
    return out
